# revision 28
# baseline (speedup 1.0000x reference)
"""Bass/Trainium2 kernel for nn_CenterTOpEX (vq_codebook): per-batch 6-step
K=2 cosine k-means over [N=16384, C=512] features, data-parallel over B=8
batches on 8 NeuronCores.

Device math per core (batch b), C-major layout (FeatureT's native layout):
  - labels_i = relu(sign(F^T w_i)),  w_i = cn1 - cn0 (normalized centers)
  - s1 = sum_{label=1} F[:, n]  via fused tensor_tensor_reduce against a
    TensorE-broadcast label row; counts via relu's accum_out
  - new centers from s1 / sum_all; norms/dists only in the last pass.
Host does the tiny post-processing (onehot, Weight scaling, means, cinidist).
"""

import numpy as np

B = 8
C = 512
N = 16384
K = 2
NQ = 4          # C // 128 partition chunks
P = 128
NT = 512        # columns per tile
T = N // NT     # 32 tiles
ITERS = 6
R_TILES = 16    # tiles resident in SBUF across iterations (rest streamed)

_BUILT = None


def _patch_tile_drain():
    """The staged walrus rejects Tile's final drain when it carries more than
    one sync wait ("Too many sync wait commands"). Split the waits across
    single-wait nops ahead of the drain."""
    import concourse.mybir as mybir
    import concourse.tile as tile
    from concourse.vector_clock import ScopedClock

    if getattr(tile.TileContext, "_ant_drain_patched", False):
        return

    def _drain_and_barrier(self, tick_clock, wait_clock):
        nc_ = self.nc
        nop0 = nc_.sync.nop()
        wait_clock.add_sem_waits(
            nop0.ins, ScopedClock({None: tick_clock.global_clock}))
        waits = list(nop0.ins.sync_info.on_wait) if nop0.ins.sync_info else []
        if len(waits) > 1:
            nop0.ins.sync_info = mybir.SyncInfo(on_wait=waits[:1], on_update=[])
            for w in waits[1:]:
                extra = nc_.sync.nop()
                extra.ins.sync_info = mybir.SyncInfo(on_wait=[w], on_update=[])
        nc_.sync.drain()
        nc_.all_engine_barrier()
        popped = nc_._tile_sem_poison_stack.pop()
        assert popped is self._sem_poison
        # This walrus also rejects wide EVENT_SEMAPHORE_RANGE_CLEAR ranges
        # ("ISA wrong length") — clear in chunks of <= 3 sems.
        sems_list = list(self.sems.allocated().values())
        for i in range(0, len(sems_list), 3):
            nc_.clear_and_free_semaphores(sems_list[i:i + 3])
        nc_.all_engine_barrier()

    tile.TileContext._drain_and_barrier = _drain_and_barrier
    tile.TileContext._ant_drain_patched = True


def _split_excess_waits(nc, max_waits=1):
    """The staged walrus accepts at most one sync wait per instruction.
    Move extra waits onto same-engine nops placed directly before the
    instruction (same per-engine program position => identical ordering).
    Nops are created through the engine API (well-formed + registered),
    then relocated from the current bb to the target position."""
    import concourse.mybir as mybir

    cur_list = nc.cur_bb.bb.instructions

    def make_nop(engine_type):
        bi = nc.engines[engine_type].nop()
        ins_obj = bi.ins
        assert cur_list[-1] is ins_obj
        cur_list.pop()
        return ins_obj

    n_split = 0
    for f in nc.m.functions:
        for blk in f.blocks:
            rebuilt = []
            changed = False
            for ins in blk.instructions:
                si = ins.sync_info
                waits = list(si.on_wait) if si and si.on_wait else []
                if len(waits) > max_waits:
                    changed = True
                    for w in waits[:-max_waits]:
                        n_split += 1
                        nop = make_nop(ins.engine)
                        nop.sync_info = mybir.SyncInfo(
                            on_wait=[w], on_update=[])
                        rebuilt.append(nop)
                    ins.sync_info = mybir.SyncInfo(
                        on_wait=waits[-max_waits:],
                        on_update=list(si.on_update) if si.on_update else [])
                rebuilt.append(ins)
            if changed:
                blk.instructions[:] = rebuilt
    return n_split


def _build(r_tiles=R_TILES):
    import concourse.bass as bass
    import concourse.mybir as mybir
    import concourse.tile as tile

    _patch_tile_drain()

    f32 = mybir.dt.float32
    f32r = mybir.dt.float32r
    AX = mybir.AxisListType.X
    OP = mybir.AluOpType
    ACT = mybir.ActivationFunctionType

    nc = bass.Bass("TRN2", target_bir_lowering=False, debug=False,
                   enable_asserts=False)

    F = nc.dram_tensor("feat", [C, N], f32, kind="ExternalInput").ap()
    CI = nc.dram_tensor("cinit", [P, NQ, K], f32, kind="ExternalInput").ap()
    C0 = nc.dram_tensor("centers0", [P, NQ, K], f32, kind="ExternalOutput").ap()
    CF = nc.dram_tensor("centersF", [P, NQ, K], f32, kind="ExternalOutput").ap()
    L0 = nc.dram_tensor("labels0", [1, N], f32, kind="ExternalOutput").ap()
    LF = nc.dram_tensor("labelsF", [1, N], f32, kind="ExternalOutput").ap()
    GF = nc.dram_tensor("gF", [K, N], f32, kind="ExternalOutput").ap()
    NSQ = nc.dram_tensor("nsq", [1, N], f32, kind="ExternalOutput").ap()

    Fv = F.rearrange("(q p) n -> p q n", p=P)  # [128, 4, N] view of HBM

    with tile.TileContext(nc) as tc:
        with (
            tc.tile_pool(name="res", bufs=1) as res_pool,
            tc.tile_pool(name="stream", bufs=3) as stream_pool,
            tc.tile_pool(name="small", bufs=1) as small,
            tc.tile_pool(name="lab", bufs=3) as lab_pool,
            tc.tile_pool(name="i5", bufs=1) as i5_pool,
            tc.tile_pool(name="junkp", bufs=1) as junkp,
            tc.tile_pool(name="pg", bufs=5, space="PSUM") as pg_pool,
            tc.tile_pool(name="plb", bufs=2, space="PSUM") as plb_pool,
            tc.tile_pool(name="psmall", bufs=1, space="PSUM") as psmall,
        ):
            # ---- constants / persistent state ----
            ones_1p = small.tile([1, P], f32, tag="ones_1p")    # bcast lhsT
            nc.vector.memset(ones_1p[:], 1.0)
            ones_p1 = small.tile([P, 1], f32, tag="ones_p1")    # colsum lhsT
            nc.vector.memset(ones_p1[:], 1.0)
            # fp32r twins: exact for 0/1 payloads, 4x faster through the PE
            # (walrus rejects f32r memset; produce via ACT copy instead)
            ones_1p_r = small.tile([1, P], f32r, tag="ones_1p_r")
            nc.scalar.copy(ones_1p_r[:], ones_1p[:])
            ones_p1_r = small.tile([P, 1], f32r, tag="ones_p1_r")
            nc.scalar.copy(ones_p1_r[:], ones_p1[:])

            centers = small.tile([P, NQ, K], f32, tag="centers")
            cnw = small.tile([P, NQ, 3], f32, tag="cnw")  # [w, cn0, cn1]
            s1acc = small.tile([P, NQ], f32, tag="s1acc")
            s1part = small.tile([P, NQ, T], f32, tag="s1part")
            sumall = small.tile([P, NQ], f32, tag="sumall")
            sumallpart = small.tile([P, NQ, T], f32, tag="sumallpart")
            cnts = small.tile([1, T], f32, tag="cnts")
            cnt1 = small.tile([1, 2], f32, tag="cnt1")   # [cnt1, n-cnt1]
            inv01 = small.tile([1, 2], f32, tag="inv01")
            invsb = small.tile([P, 2], f32, tag="invsb")
            ssq = small.tile([1, K], f32, tag="ssq")
            nrm = small.tile([1, K], f32, tag="nrm")
            nrmr = small.tile([P, K], f32, tag="nrmr")
            junk = junkp.tile([P, NT], f32, tag="junk")

            nc.sync.dma_start(out=centers[:], in_=CI[:])

            def normalize_centers():
                """centers -> cnw = [cn0, cn1, cn1-cn0] (normalize rows)."""
                sqc = small.tile([P, NQ, K], f32, tag="sqc")
                nc.scalar.activation(sqc[:], centers[:], ACT.Square)
                pcs = psmall.tile([128, 8], f32, tag="psm", name="pcs")[0:1, 0:NQ * K]
                nc.tensor.matmul(pcs[:], ones_p1[:], sqc[:].rearrange("p q k -> p (q k)"),
                                 start=True, stop=True)
                # sum over q per k: view [1, (q k)] -> [1, k, q]
                nc.vector.reduce_sum(ssq[:], pcs[:].rearrange("p (q k) -> p k q", k=K),
                                     axis=AX)
                nc.scalar.sqrt(nrm[:], ssq[:])
                nc.vector.tensor_scalar_max(nrm[:], nrm[:], 1e-12)
                rcp = small.tile([1, K], f32, tag="rcp")
                nc.vector.reciprocal(rcp[:], nrm[:])
                prep = psmall.tile([128, 8], f32, tag="psm", name="prep")[:, 0:K]
                nc.tensor.matmul(prep[:], ones_1p[:], rcp[:], start=True, stop=True)
                nc.scalar.copy(nrmr[:], prep[:])
                for k in range(K):
                    nc.vector.tensor_scalar(cnw[:, :, 1 + k], centers[:, :, k],
                                            nrmr[:, k:k + 1], None, op0=OP.mult)
                nc.vector.tensor_sub(cnw[:, :, 0], cnw[:, :, 2], cnw[:, :, 1])

            normalize_centers()

            # resident tiles (loaded during iteration 0, reused after)
            rtiles = [res_pool.tile([P, NQ, NT], f32, tag=f"res{t}",
                                    name=f"res{t}")
                      for t in range(r_tiles)]

            # interleave resident/streamed for DMA overlap
            order = []
            rs, ss = list(range(r_tiles)), list(range(r_tiles, T))
            for i in range(T):
                if i % 2 == 0 and ss:
                    order.append(ss.pop(0))
                elif rs:
                    order.append(rs.pop(0))
                else:
                    order.append(ss.pop(0))

            for it in range(ITERS):
                last = it == ITERS - 1
                for tix, t in enumerate(order):
                    n0 = t * NT
                    if it == 0 and t < r_tiles:
                        ft = rtiles[t]
                        nc.sync.dma_start(out=ft[:], in_=Fv[:, :, n0:n0 + NT])
                    elif t < r_tiles:
                        ft = rtiles[t]
                    else:
                        ft = stream_pool.tile([P, NQ, NT], f32, tag="stream")
                        nc.sync.dma_start(out=ft[:], in_=Fv[:, :, n0:n0 + NT])

                    lab = lab_pool.tile([1, NT], f32r, tag="lab")
                    if not last:
                        pgw = pg_pool.tile([2, NT], f32, tag="pg", name="pgw")[0:1, :]
                        for q in range(NQ):
                            nc.tensor.matmul(pgw[:], cnw[:, q, 0:1], ft[:, q, :],
                                             start=(q == 0), stop=(q == NQ - 1))
                        sg = lab_pool.tile([1, NT], f32, tag="sg")
                        nc.scalar.sign(sg[:], pgw[:])
                        nc.scalar.activation(lab[:], sg[:], ACT.Relu,
                                             accum_out=cnts[:, t:t + 1])
                    else:
                        pgg = pg_pool.tile([2, NT], f32, tag="pg", name="pgg")
                        for q in range(NQ):
                            nc.tensor.matmul(pgg[:], cnw[:, q, 1:3], ft[:, q, :],
                                             start=(q == 0), stop=(q == NQ - 1))
                        gg = lab_pool.tile([2, NT], f32, tag="gg")
                        nc.scalar.copy(gg[:], pgg[:])
                        nc.sync.dma_start(out=GF[:, n0:n0 + NT], in_=gg[:])
                        g1r = lab_pool.tile([1, NT], f32, tag="g1r")
                        nc.sync.dma_start(out=g1r[:], in_=gg[1:2, :])
                        # label = 1 iff g1 > g0 (tie -> 0, matches argmin)
                        nc.vector.scalar_tensor_tensor(
                            out=lab[:], in0=g1r[:], scalar=1.0, in1=gg[0:1, :],
                            op0=OP.mult, op1=OP.is_gt,
                            accum_out=cnts[:, t:t + 1])

                    plb = plb_pool.tile([P, NT], f32, tag="plb")
                    nc.tensor.matmul(plb[:], ones_1p_r[:], lab[:], start=True, stop=True)

                    for q in range(NQ):
                        nc.vector.scalar_tensor_tensor(
                            out=junk[:], in0=ft[:, q, :], scalar=1.0,
                            in1=plb[:], op0=OP.mult, op1=OP.mult,
                            accum_out=s1part[:, q, t:t + 1])

                    if it == 0:
                        for q in range(NQ):
                            nc.vector.reduce_sum(
                                sumallpart[:, q, t:t + 1], ft[:, q, :], axis=AX)
                        nc.sync.dma_start(out=L0[0:1, n0:n0 + NT],
                                          in_=lab[:].bitcast(f32))
                        sq = i5_pool.tile([P, NQ, NT], f32r, tag="sq")
                        nc.scalar.activation(sq[:], ft[:], ACT.Square)
                        pn = pg_pool.tile([2, NT], f32, tag="pg", name="pn")[0:1, :]
                        for q in range(NQ):
                            nc.tensor.matmul(pn[:], ones_p1_r[:], sq[:, q, :],
                                             start=(q == 0), stop=(q == NQ - 1))
                        nsqs = lab_pool.tile([1, NT], f32, tag="nsqs")
                        nc.scalar.copy(nsqs[:], pn[:])
                        nc.sync.dma_start(out=NSQ[0:1, n0:n0 + NT], in_=nsqs[:])

                    if last:
                        nc.sync.dma_start(out=LF[0:1, n0:n0 + NT],
                                          in_=lab[:].bitcast(f32))

                # ---- end of pass: counts, new centers ----
                nc.vector.reduce_sum(s1acc[:, :], s1part[:, :, :], axis=AX)
                if it == 0:
                    nc.vector.reduce_sum(sumall[:, :], sumallpart[:, :, :], axis=AX)
                if it == 0 or last:
                    # exported centers need the true /(cnt+1) scale
                    nc.vector.reduce_sum(cnt1[:, 0:1], cnts[:], axis=AX)
                    nc.vector.tensor_scalar(cnt1[:, 1:2], cnt1[:, 0:1], -1.0,
                                            float(N), op0=OP.mult, op1=OP.add)
                    nc.vector.tensor_scalar_add(inv01[:], cnt1[:], 1.0)
                    nc.vector.reciprocal(inv01[:], inv01[:])
                    pinv = psmall.tile([128, 8], f32, tag="psm", name="pinv")[:, 0:2]
                    nc.tensor.matmul(pinv[:], ones_1p[:], inv01[:],
                                     start=True, stop=True)
                    nc.scalar.copy(invsb[:], pinv[:])
                    nc.vector.tensor_scalar(centers[:, :, 1], s1acc[:, :],
                                            invsb[:, 0:1], None, op0=OP.mult)
                    nc.vector.scalar_tensor_tensor(
                        out=centers[:, :, 0], in0=s1acc[:, :], scalar=-1.0,
                        in1=sumall[:, :], op0=OP.mult, op1=OP.add)
                    nc.vector.tensor_scalar(centers[:, :, 0], centers[:, :, 0],
                                            invsb[:, 1:2], None, op0=OP.mult)
                else:
                    # interior iters: row-normalize cancels any per-row scale,
                    # so skip the count-divide and shorten the boundary chain
                    nc.scalar.copy(centers[:, :, 1], s1acc[:, :])
                    nc.vector.scalar_tensor_tensor(
                        out=centers[:, :, 0], in0=s1acc[:, :], scalar=-1.0,
                        in1=sumall[:, :], op0=OP.mult, op1=OP.add)
                if it == 0:
                    nc.sync.dma_start(out=C0[:], in_=centers[:])
                if last:
                    nc.sync.dma_start(out=CF[:], in_=centers[:])
                else:
                    normalize_centers()

    _split_excess_waits(nc)
    return nc


def _get_nc():
    global _BUILT
    if _BUILT is None:
        _BUILT = _build()
    return _BUILT


def _dec_centers(dev):
    # dev [128, 4, 2] with dev[p, q, k] = centers[k, q*128 + p]
    return np.ascontiguousarray(dev.transpose(2, 1, 0).reshape(K, C))


def kernel(FeatureT, centerInit):
    from concourse.bass_utils import run_bass_kernel_spmd

    FeatureT = np.asarray(FeatureT)
    centerInit = np.asarray(centerInit)
    feats = FeatureT.reshape(B, C, N)
    ci_dev = np.ascontiguousarray(
        centerInit.T.reshape(NQ, P, K).transpose(1, 0, 2))

    nc = _get_nc()
    in_maps = [{"feat": np.ascontiguousarray(feats[b]), "cinit": ci_dev}
               for b in range(B)]
    res = run_bass_kernel_spmd(nc, in_maps, core_ids=list(range(B))).results

    c0 = np.stack([_dec_centers(res[b]["centers0"]) for b in range(B)])
    cF = np.stack([_dec_centers(res[b]["centersF"]) for b in range(B)])
    labelsF = np.stack([res[b]["labelsF"][0] for b in range(B)])
    labels0 = np.stack([res[b]["labels0"][0] for b in range(B)])
    g = np.stack([res[b]["gF"] for b in range(B)])          # [B, 2, N]
    nsq = np.stack([res[b]["nsq"][0] for b in range(B)])    # [B, N]
    norm = np.maximum(np.sqrt(nsq), np.float32(1e-12))
    dists = (0.5 * (1.0 - g / norm[:, None, :])).astype(np.float32)

    centersIterout = cF.mean(axis=0)
    labelsout = labelsF.astype(np.int32)
    labelPinit = labels0.astype(np.int32)
    onehot = np.stack([1.0 - labelsF, labelsF], axis=-1).astype(np.float32)

    d = np.ascontiguousarray(dists.transpose(0, 2, 1))  # [B, N, 2]
    dmin = d.min(axis=1, keepdims=True)
    dmax = d.max(axis=1, keepdims=True)
    Weight = (1.0 - (d - dmin) / (dmax - dmin + np.float32(1e-7))).astype(np.float32)

    # cinidist: mean_k cos_sim(centers_after_iter0, centerInit), mean over b
    a = c0  # [B, K, C]
    bb = np.broadcast_to(centerInit, (B, K, C))
    na = np.maximum(np.linalg.norm(a, axis=-1), 1e-8)
    nb = np.maximum(np.linalg.norm(bb, axis=-1), 1e-8)
    cos = (a * bb).sum(axis=-1) / (na * nb)  # [B, K]
    Cinidist = np.float32(cos.mean())

    return (centersIterout.astype(np.float32), labelsout, onehot, Weight,
            labelPinit, Cinidist)
